# revision 1
# baseline (speedup 1.0000x reference)
"""MaxPool3d (kernel=2, stride=2) on Trainium2, 8-core data-parallel.

Input  x: (2, 32, 128, 128, 128) f32  -> flattened to 64 channels, 8 per core.
Output y: (2, 32, 64, 64, 64) f32.

Per-core layout: one tile covers half a channel (64 input D-planes).
SBUF partition p = (d' in 0..31, hb in 0..3) where d' = output depth index
within the tile and hb = quarter-of-H block; p = 4*d' + hb is affine over
DRAM (partition stride = 4096 elements = 16 KiB contiguous run per
partition), so every DMA spans all 128 partitions with large contiguous
descriptors.

Pooling = three cascaded elementwise-max stages on VectorE, all within the
free dimension:
  1. D-pairs:  max(A0, A1) where A0/A1 are the even/odd plane loads
  2. H-pairs:  max over row pairs (step-2 row slices)
  3. W-pairs:  max over element pairs (step-2 element slices)
"""

import numpy as np

import concourse.bass as bass
import concourse.tile as tile
from concourse import bacc, mybir
from concourse import bass_utils

CPC = 8            # channels per core (64 total B*C over 8 cores)
D = H = W = 128
DT = mybir.dt.float32

_CACHE = {}


def _build_module():
    nc = bacc.Bacc("TRN2", target_bir_lowering=False, debug=False, num_devices=8)
    x = nc.dram_tensor("x", [CPC, D, H, W], DT, kind="ExternalInput").ap()
    y = nc.dram_tensor("y", [CPC, D // 2, H // 2, W // 2], DT, kind="ExternalOutput").ap()

    with tile.TileContext(nc) as tc:
        with tc.tile_pool(name="loads", bufs=2) as loads, \
             tc.tile_pool(name="work", bufs=2) as work:
            for c in range(CPC):
                for half in range(2):
                    base = half * 64  # input D-plane base for this tile
                    # [128p = (d':32, hb:4), 32 rows, 128 w] per plane-parity
                    a0 = loads.tile([128, 32, 128], DT, name="a0")
                    nc.sync.dma_start(a0, x[c, base : base + 64 : 2])
                    a1 = loads.tile([128, 32, 128], DT, name="a1")
                    nc.sync.dma_start(a1, x[c, base + 1 : base + 64 : 2])

                    dmax = work.tile([128, 32, 128], DT, name="dmax")
                    nc.vector.tensor_max(dmax, a0, a1)

                    hmax = work.tile([128, 16, 128], DT, name="hmax")
                    nc.vector.tensor_max(hmax, dmax[:, 0::2, :], dmax[:, 1::2, :])

                    wpair = hmax.rearrange("p r (w2 t) -> p r w2 t", t=2)
                    wmax = work.tile([128, 16, 64], DT, name="wmax")
                    nc.vector.tensor_max(wmax, wpair[:, :, :, 0], wpair[:, :, :, 1])

                    nc.scalar.dma_start(y[c, half * 32 : half * 32 + 32], wmax)

    nc.compile()
    return nc


def _get_module():
    if "nc" not in _CACHE:
        _CACHE["nc"] = _build_module()
    return _CACHE["nc"]


def kernel(x: np.ndarray) -> np.ndarray:
    B, C, d, h, w = x.shape
    assert (B, C, d, h, w) == (2, 32, 128, 128, 128), x.shape
    nc = _get_module()

    xf = np.ascontiguousarray(x, dtype=np.float32).reshape(B * C, d, h, w)
    in_maps = [
        {"x": np.ascontiguousarray(xf[i * CPC : (i + 1) * CPC])} for i in range(8)
    ]
    res = bass_utils.run_bass_kernel_spmd(nc, in_maps, core_ids=list(range(8)))
    out = np.concatenate([r["y"] for r in res.results], axis=0)
    return out.reshape(B, C, d // 2, h // 2, w // 2)


# revision 2
# speedup vs baseline: 1.0097x; 1.0097x over previous
"""MaxPool3d (kernel=2, stride=2) on Trainium2, 8-core data-parallel.

Input  x: (2, 32, 128, 128, 128) f32  -> flattened to 64 channels, 8 per core.
Output y: (2, 32, 64, 64, 64) f32.

Per-core layout: one tile covers half a channel (64 input D-planes).
SBUF partition p = (d' in 0..31, hb in 0..3) where d' = output depth index
within the tile and hb = quarter-of-H block; p = 4*d' + hb is affine over
DRAM (partition stride = 4096 elements = 16 KiB contiguous run per
partition), so every DMA spans all 128 partitions with large contiguous
descriptors.

Pooling = three cascaded elementwise-max stages on VectorE, all within the
free dimension:
  1. D-pairs:  max(A0, A1) where A0/A1 are the even/odd plane loads
  2. H-pairs:  max over row pairs (step-2 row slices)
  3. W-pairs:  max over element pairs (step-2 element slices)
"""

import numpy as np

import concourse.bass as bass
import concourse.tile as tile
from concourse import bacc, mybir
from concourse import bass_utils

CPC = 8            # channels per core (64 total B*C over 8 cores)
D = H = W = 128
DT = mybir.dt.float32

_CACHE = {}


def _build_module():
    nc = bacc.Bacc("TRN2", target_bir_lowering=False, debug=False, num_devices=8)
    x = nc.dram_tensor("x", [CPC, D, H, W], DT, kind="ExternalInput").ap()
    y = nc.dram_tensor("y", [CPC, D // 2, H // 2, W // 2], DT, kind="ExternalOutput").ap()

    with tile.TileContext(nc) as tc:
        with tc.tile_pool(name="loads", bufs=3) as loads, \
             tc.tile_pool(name="work", bufs=3) as work:
            for c in range(CPC):
                for half in range(2):
                    base = half * 64  # input D-plane base for this tile
                    # [128p = (d':32, hb:4), 32 rows, 128 w] per plane-parity
                    a0 = loads.tile([128, 32, 128], DT, name="a0")
                    nc.sync.dma_start(a0, x[c, base : base + 64 : 2])
                    a1 = loads.tile([128, 32, 128], DT, name="a1")
                    nc.sync.dma_start(a1, x[c, base + 1 : base + 64 : 2])

                    dmax = work.tile([128, 32, 128], DT, name="dmax")
                    nc.vector.tensor_max(dmax, a0, a1)

                    hmax = work.tile([128, 16, 128], DT, name="hmax")
                    nc.vector.tensor_max(hmax, dmax[:, 0::2, :], dmax[:, 1::2, :])

                    wpair = hmax.rearrange("p r (w2 t) -> p r w2 t", t=2)
                    wmax = work.tile([128, 16, 64], DT, name="wmax")
                    nc.vector.tensor_max(wmax, wpair[:, :, :, 0], wpair[:, :, :, 1])

                    nc.scalar.dma_start(y[c, half * 32 : half * 32 + 32], wmax)

    nc.compile()
    return nc


def _get_module():
    if "nc" not in _CACHE:
        _CACHE["nc"] = _build_module()
    return _CACHE["nc"]


def kernel(x: np.ndarray) -> np.ndarray:
    B, C, d, h, w = x.shape
    assert (B, C, d, h, w) == (2, 32, 128, 128, 128), x.shape
    nc = _get_module()

    xf = np.ascontiguousarray(x, dtype=np.float32).reshape(B * C, d, h, w)
    in_maps = [
        {"x": np.ascontiguousarray(xf[i * CPC : (i + 1) * CPC])} for i in range(8)
    ]
    res = bass_utils.run_bass_kernel_spmd(nc, in_maps, core_ids=list(range(8)))
    out = np.concatenate([r["y"] for r in res.results], axis=0)
    return out.reshape(B, C, d // 2, h // 2, w // 2)
